# revision 121
# baseline (speedup 1.0000x reference)
"""Multi-head causal attention (B=4, S=2048, D=1024, H=16) on 8 trn2 NeuronCores.

Sharding: data-parallel over batch (4) x tensor-parallel over heads (2 groups
of 8).  Core c = (b, g) computes, for batch b, head group g:
  Projections run as fp8e4m3 DoubleRow matmuls (256 contraction rows per
  matmul at 0.5 cyc/output-elem = 4x fp16 PE throughput), with residual
  splits sized to the error budget (host precomputes all fp8 operands):
    Q^T = e4m3(Xh Wq8 + Xl Wq8 + bq)   2-term: X = Xh + Xl exact, Wq single
                                        fp8 (only W-quant noise ~3%)
    K^T likewise 2-term, then split into fp8 hi/lo pair (repr of K exact)
    V   = (Xh + Xl) Wv_h + Xh Wv_l32 / 32   3-term: separate psum group for
          the x32-scaled W-residual (W ~ +-1/32 underflows e4m3 unscaled),
          combined + cast f16 by one DVE scalar_tensor_tensor.  NO bias (bv
          contributes bv @ Wo, folded into the host-side bias).  A ones
          column is appended for softmax row sums.
  per head h, query tile qt (128):
     S^T[k, q] = (K_hi,K_lo)^T (Q̂,Q̂)   one fp8 DoubleRow matmul per k-tile
     diagonal tile: extra DR matmul accumulates -448*step[k,q] into psum so
       masked scores exp to exactly 0 (no post-exp masking pass)
     A^T = fp16(exp(S^T / 8))          (ScalarE, k-tiles packed in psum,
                                        one instruction covers both heads;
                                        for qt >= 9 the first 4-ktile block
                                        runs on DVE instead as an fp16
                                        Schraudolph bit-trick exp --
                                        bitcast_f16(int16(s*a + b)) -- whose
                                        ~1.8% centered noise the row
                                        normalization absorbs, offloading
                                        the otherwise-saturated ScalarE)
     psum_o[q, 65] += A^T.T V          fp16 matmuls, col 64 = row sums via
                                        the ones column
     O[q, h*64:+64] = psum_o[:, :64] * recip(psum_o[:, 64])  (one DVE
       tensor_tensor per head pair, broadcast recips, psum -> f16)
  O^T via SBUF-to-SBUF XBAR DMA transpose (the final q-tile instead uses
  an on-PE identity transpose + ScalarE copy, skipping the DMA HWDGE-gen
  + queue + sem latency on the critical tail), out^T = Wo_g^T O^T (fp16)
Host sums the two per-batch partials and adds bo + bv @ Wo.

Scheduling: each attention call's mm2+normalize is deferred until after
the NEXT call's first mm1/exp block so ScalarE/DVE stream exps without
waiting on PE tail work; X chunks prefetch one chunk ahead through
dedicated SBUF rings; the Xv/32 operand for the V lo-term is derived on
the idle Pool engine rather than transferred.
"""

import math

import numpy as np

B, S, D, H = 4, 2048, 1024, 16
HD = D // H          # 64
NCORES = 8
HPC = 8              # heads per core
DM = HPC * HD        # 512 mid-dims per core
NQT = S // 128       # 16 query tiles
KT_PER_EXP = 4       # k-tiles per head packed into one [128, 1024] psum before exp
VROW = 65            # per-head V columns incl. ones column

_CACHE = {}


def _build_program():
    import concourse.mybir as mybir
    import concourse.tile as tile
    from concourse import bacc

    f32 = mybir.dt.float32
    f16 = mybir.dt.float16
    i16 = mybir.dt.int16
    e4m = mybir.dt.float8e4
    SCH_QT = 9                       # offload exp blk0 to DVE for qt >= this
    SCH_A = 1024.0 / math.log(2.0) / 8.0
    SCH_B = 15360.0 - 60.0
    EXP = mybir.ActivationFunctionType.Exp
    IDENT = mybir.ActivationFunctionType.Identity
    COPY = mybir.ActivationFunctionType.Copy
    DR = mybir.MatmulPerfMode.DoubleRow
    ADD = mybir.AluOpType.add
    SUB = mybir.AluOpType.subtract
    MUL = mybir.AluOpType.mult

    nc = bacc.Bacc("TRN2", target_bir_lowering=False, debug=False,
                   num_devices=NCORES)

    xqT_d = nc.dram_tensor("xqT", [2 * D, S], e4m, kind="ExternalInput")
    xkT_d = nc.dram_tensor("xkT", [2 * D, S], e4m, kind="ExternalInput")
    xvT_d = nc.dram_tensor("xvT", [2 * D, S], e4m, kind="ExternalInput")
    wq_d = nc.dram_tensor("wq", [D, DM], e4m, kind="ExternalInput")
    wk_d = nc.dram_tensor("wk", [D, DM], e4m, kind="ExternalInput")
    wv_d = nc.dram_tensor("wv", [2 * D, DM], e4m, kind="ExternalInput")
    bq_d = nc.dram_tensor("bq", [128, 4], f32, kind="ExternalInput")
    bk_d = nc.dram_tensor("bk", [128, 4], f32, kind="ExternalInput")
    wo_d = nc.dram_tensor("wo", [DM, D], f16, kind="ExternalInput")
    maskw_d = nc.dram_tensor("maskw", [128, 256], e4m, kind="ExternalInput")
    maskr_d = nc.dram_tensor("maskr", [128, 256], e4m, kind="ExternalInput")
    ident_d = nc.dram_tensor("ident", [128, 128], f16, kind="ExternalInput")
    outT_d = nc.dram_tensor("outT", [D, S], f16, kind="ExternalOutput")

    with tile.TileContext(nc) as tc:
        with (
            tc.tile_pool(name="res", bufs=1) as res,     # long-lived tensors
            tc.tile_pool(name="wrk", bufs=1) as wrk,     # rotating work tiles
            tc.tile_pool(name="ps", bufs=1, space="PSUM") as ps,
        ):
            # ---- resident tensors -------------------------------------
            wq_sb = res.tile([128, 8 * DM], e4m, tag="wq_sb")
            wk_sb = res.tile([128, 8 * DM], e4m, tag="wk_sb")
            wv_sb = res.tile([128, 16 * DM], e4m, tag="wv_sb")
            wo_sb = res.tile([128, 4 * D], f16, tag="wo_sb")
            bq_sb = res.tile([128, 4], f32, tag="bq_sb")
            bk_sb = res.tile([128, 4], f32, tag="bk_sb")
            maskw_sb = res.tile([128, 256], e4m, tag="maskw_sb")
            maskr_sb = res.tile([128, 256], e4m, tag="maskr_sb")
            ident_sb = res.tile([128, 128], f16, tag="ident_sb")
            qT_sb = [res.tile([128, S], e4m, tag=f"qT{m}", name=f"qT{m}")
                     for m in range(4)]
            kT_sb = [res.tile([128, 2 * S], e4m, tag=f"kT{m}", name=f"kT{m}")
                     for m in range(4)]
            v_sb = res.tile([128, NQT * HPC * VROW], f16, tag="v_sb")
            oT_sb = res.tile([128, 4 * S], f16, tag="oT_sb")

            kT_v = [t.rearrange("p (t s) -> p t s", t=2) for t in kT_sb]
            # [p, ktile, head, hi/lo, col]
            v5 = v_sb.rearrange("p (s h c) -> p s h c", h=HPC, c=VROW)
            oT4 = oT_sb.rearrange("p (k s) -> p k s", k=4)
            # per-head views at the head's partition base: a psum accumulation
            # group whose matmuls sit on different row strips (score at base
            # 64, mask at base 0) wedges the device, so head 1 reads an
            # identical mask copy from partitions 64-127
            maskw_v = [maskw_sb[p0 : p0 + 64, :].rearrange("p (t k) -> p t k", t=2)
                       for p0 in (0, 64)]
            maskr_v = [maskr_sb[p0 : p0 + 64, :].rearrange("p (t k) -> p t k", t=2)
                       for p0 in (0, 64)]

            def load_w(w_sb, w_d, n_w, cols=None):
                sv = w_sb.rearrange("p (k n) -> p k n", n=n_w)
                dv = w_d.rearrange("(k p) n -> p k n", p=128)
                if cols is not None:
                    sv = sv[:, :, cols[0] : cols[1]]
                    dv = dv[:, :, cols[0] : cols[1]]
                nc.sync.dma_start(sv, dv)

            # ---- projections, split into per-m-tile units -------------
            # xch holds fp8 hi chunks (k=0..7) then lo chunks (k=8..15); DR
            # matmuls pair adjacent 128-row chunks (256 contraction rows per
            # matmul at 0.5 cyc/output-elem)
            def load_xch(xT_d, n, nk=16, tag="xch"):
                xch = wrk.tile([128, nk * 512], e4m, tag=tag, name=tag,
                               bufs=2)
                sv = xch.rearrange("p (k s) -> p k s", k=nk)
                dv = xT_d.rearrange("(k p) s -> p k s", p=128)[
                    :, :, n * 512 : (n + 1) * 512
                ]
                if nk == 24:
                    # V chunk: DMA only Xh/Xl (k 0-15); Xh/32 (k 16-23) is
                    # computed on the idle Pool engine, per mi strip
                    nc.sync.dma_start(sv[:, 0:16, :], dv[:, 0:16, :])
                else:
                    nc.sync.dma_start(sv, dv)
                return xch

            def fill_h32(xch, mi):
                xv_ = xch.rearrange("p (k s) -> p k s", k=24)
                sl = slice(mi * 128, (mi + 1) * 128)
                nc.gpsimd.tensor_scalar_mul(
                    xv_[:, 16:24, sl], xv_[:, 0:8, sl], 1.0 / 32.0,
                )

            def proj_mm(xch, w_sb, m):
                # 2-term: (Xh + Xl) @ W8, one psum accumulation group
                pp = ps.tile([128, 512], f32, tag="ps_small", name="pp",
                             bufs=2)
                wv_ = w_sb.rearrange("p (k n) -> p k n", n=DM)
                xv_ = xch.rearrange("p (k s) -> p k s", k=16)
                lhs = wv_[:, :, m * 128 : (m + 1) * 128]
                idx = 0
                for base in (0, 8):
                    for j in range(4):
                        nc.tensor.matmul(
                            pp[:],
                            lhs[:, 2 * j : 2 * j + 2, :],
                            xv_[:, base + 2 * j : base + 2 * j + 2, :],
                            start=(idx == 0),
                            stop=(idx == 7),
                            perf_mode=DR,
                        )
                        idx += 1
                return pp

            def proj_q_unit(xch, n, m):
                pp = proj_mm(xch, wq_sb, m)
                nc.vector.tensor_scalar_add(
                    qT_sb[m][:, n * 512 : (n + 1) * 512], pp[:],
                    bq_sb[:, m : m + 1],
                )

            def proj_k_unit(xch, n, m, fast0=False):
                pp = proj_mm(xch, wk_sb, m)
                # fast0: split the bias-add + hi/lo pair so k-tile 0 (the
                # only tile attention(hp, 0) needs) is ready ~1.3us sooner
                # at kernel start
                cuts = ((0, 128), (128, 512)) if fast0 else ((0, 512),)
                for c0, c1 in cuts:
                    sl = slice(n * 512 + c0, n * 512 + c1)
                    nc.vector.tensor_scalar_add(
                        kT_v[m][:, 0, sl], pp[:, c0:c1], bk_sb[:, m : m + 1],
                    )
                    nc.vector.scalar_tensor_tensor(
                        kT_v[m][:, 1, sl], pp[:, c0:c1], bk_sb[:, m : m + 1],
                        kT_v[m][:, 0, sl], op0=ADD, op1=SUB,
                    )

            def proj_v_unit(xch, n, mi):
                st = n * 4 + mi          # S tile index
                # 3-term, one psum group: (Xh+Xl) Wh + (Xh/32)(32 Wl).
                # xch chunk layout: 0-7 Xh, 8-15 Xl, 16-23 Xh/32.
                pp = ps.tile([128, 512], f32, tag="ps_small", name="pp",
                             bufs=2)
                xv_ = xch.rearrange("p (k s) -> p k s", k=24)
                wvv = wv_sb.rearrange("p (k n) -> p k n", n=DM)
                lhs = xv_[:, :, mi * 128 : (mi + 1) * 128]
                idx = 0
                for base, wb in ((0, 0), (8, 0), (16, 8)):
                    for j in range(4):
                        nc.tensor.matmul(
                            pp[:],
                            lhs[:, base + 2 * j : base + 2 * j + 2, :],
                            wvv[:, wb + 2 * j : wb + 2 * j + 2, :],
                            start=(idx == 0),
                            stop=(idx == 11),
                            perf_mode=DR,
                        )
                        idx += 1
                pp3 = pp.rearrange("p (h c) -> p h c", h=HPC)
                nc.vector.tensor_copy(v5[:, st, :, 0:HD], pp3)

            preloaded = {}

            def prefetch(n):
                if n > 3 or n in preloaded:
                    return
                preloaded[n] = (load_xch(xqT_d, n, tag="xchq"),
                                load_xch(xkT_d, n, tag="xchk"),
                                load_xch(xvT_d, n, nk=24, tag="xchv"))

            def proj_units(n):
                xq, xk, xv = preloaded.pop(n)
                prefetch(n + 1)
                for m in range(4):
                    yield lambda m=m, x=xq: proj_q_unit(x, n, m)
                for m in range(4):
                    yield lambda m=m, x=xk: proj_k_unit(x, n, m)
                for mi in range(4):
                    fill_h32(xv, mi)
                for mi in range(4):
                    yield lambda mi=mi, x=xv: proj_v_unit(x, n, mi)

            # ---- attention -------------------------------------------
            # pending: deferred tails (last mm2 block + normalize, and
            # transpose_o) consumed right after the NEXT attention call's
            # first mm1/exp block is issued, so ScalarE streams exps
            # back-to-back instead of waiting for PE tail work
            pending = []

            def flush_pending(limit=None):
                n = len(pending) if limit is None else min(limit, len(pending))
                for _ in range(n):
                    pending.pop(0)()

            def attention(hp, qt, o_nat, drain=None, inline=False,
                          tail_split=False, local=False):
                nblk = qt + 1
                heads = (2 * hp, 2 * hp + 1)
                aT = wrk.tile([128, 2 * S], f16, tag="aT", name="aT", bufs=6)
                aTv = aT.rearrange("p (h s q) -> p h s q", h=2, q=128)
                po = ps.tile([128, 1024], f32, tag="ps_o", name="po", bufs=1)
                pof = [po[:, 0:VROW], po[:, 512 : 512 + VROW]]

                def mm1_exp(blk, cnt):
                    psx = ps.tile([128, 2 * KT_PER_EXP * 128], f32,
                                  tag="ps_s", name="psx", bufs=2)
                    for j in range(cnt):
                        kt = blk * KT_PER_EXP + j
                        diag = kt == qt
                        for hh in range(2):
                            p0 = hh * 64
                            rhs = qT_sb[hp][
                                p0 : p0 + 64, qt * 128 : (qt + 1) * 128
                            ].unsqueeze(1).broadcast_to([64, 2, 128])
                            dst = psx[:, hh * KT_PER_EXP * 128 + j * 128 :
                                      hh * KT_PER_EXP * 128 + (j + 1) * 128]
                            nc.tensor.matmul(
                                dst,
                                kT_v[hp][p0 : p0 + 64, :,
                                         kt * 128 : (kt + 1) * 128],
                                rhs,
                                start=True,
                                stop=not diag,
                                perf_mode=DR,
                                skip_group_check=True,
                            )
                            if diag:
                                nc.tensor.matmul(
                                    dst,
                                    maskw_v[hh],
                                    maskr_v[hh],
                                    start=False,
                                    stop=True,
                                    perf_mode=DR,
                                    skip_group_check=True,
                                )
                    psxv = psx.rearrange("p (h j q) -> p h j q", h=2, q=128)
                    ksl = slice(blk * KT_PER_EXP, blk * KT_PER_EXP + cnt)
                    if blk == 0 and qt >= SCH_QT:
                        # far-from-diagonal k-tiles: fp16 Schraudolph exp on
                        # DVE (bitcast_f16(int16(s*a + b)) ~ exp(s/8), 1.8%
                        # centered noise, absorbed by the row normalization)
                        # to offload the otherwise-saturated ScalarE
                        aTi = aT[:].bitcast(i16).rearrange(
                            "p (h s q) -> p h s q", h=2, q=128)
                        with nc.allow_low_precision(reason="schraudolph exp"):
                            nc.vector.tensor_scalar(
                                aTi[:, :, ksl, :], psxv[:, :, 0:cnt, :],
                                SCH_A, SCH_B, op0=MUL, op1=ADD,
                            )
                    else:
                        nc.scalar.activation(
                            aTv[:, :, ksl, :],
                            psxv[:, :, 0:cnt, :],
                            EXP,
                            scale=1.0 / math.sqrt(HD),
                        )

                def mm2(blk, cnt):
                    for hh in range(2):
                        for j in range(cnt):
                            kt = blk * KT_PER_EXP + j
                            nc.tensor.matmul(
                                pof[hh],
                                aTv[:, hh, kt, :],
                                v5[:, kt, heads[hh], :],
                                start=(kt == 0),
                                stop=(kt == nblk - 1),
                                skip_group_check=True,
                            )

                nexp = (nblk + KT_PER_EXP - 1) // KT_PER_EXP
                cnts = [min(KT_PER_EXP, nblk - b * KT_PER_EXP)
                        for b in range(nexp)]
                for blk in range(nexp):
                    mm1_exp(blk, cnts[blk])
                    if blk == min(1, nexp - 1):
                        flush_pending(drain)
                    if inline and blk > 0:
                        mm2(blk - 1, cnts[blk - 1])

                def norm():
                    rc = wrk.tile([128, 2], f32, tag="rc", name="rc", bufs=12)
                    po_pair = po.rearrange("p (b c) -> p b c", c=512)
                    nc.vector.reciprocal(rc[:], po_pair[:, :, HD : HD + 1])
                    rcb = rc[:].unsqueeze(2).broadcast_to([128, 2, HD])
                    if local:
                        o_pair = o_nat.rearrange("p (h c) -> p h c", h=2)
                    else:
                        o_pair = o_nat.rearrange("p (h c) -> p h c", h=HPC)[
                            :, 2 * hp : 2 * hp + 2, :
                        ]
                    nc.vector.tensor_tensor(
                        o_pair, po_pair[:, :, 0:HD], rcb, op=MUL,
                    )

                def strip_transpose():
                    # per-head-pair transpose from a PER-CALL [128,128]
                    # tile (separate tiles avoid the conservative DMA-read
                    # hazard that would stall the other head-pairs' norms)
                    src_ap = o_nat[:] if local else \
                        o_nat[:, hp * 128 : (hp + 1) * 128]
                    nc.sync.dma_start_transpose(
                        oT4[:, hp, qt * 128 : (qt + 1) * 128], src_ap,
                    )

                if inline:
                    mm2(nexp - 1, cnts[nexp - 1])
                    norm()
                    strip_transpose()
                    return

                if tail_split:
                    # very last call: no future exps to protect, so run all
                    # but the final mm2 block now; only the last block +
                    # norm stay gated on the final exp
                    for blk in range(nexp - 1):
                        mm2(blk, cnts[blk])

                    def tail_s():
                        mm2(nexp - 1, cnts[nexp - 1])
                        norm()

                    pending.append(tail_s)
                    return

                def tail():
                    for blk in range(nexp):
                        mm2(blk, cnts[blk])
                    norm()

                pending.append(tail)

            def transpose_o(qt, o_nat):
                # SBUF->SBUF XBAR transpose: oT4[d, k, q] = o_nat[q, k*128+d]
                nc.sync.dma_start_transpose(
                    oT4[:, :, qt * 128 : (qt + 1) * 128], o_nat[:]
                )

            def outproj_unit(n, m8):
                pp = ps.tile([128, 512], f32, tag="ps_small", name="pp",
                             bufs=2)
                for kt in range(4):
                    nc.tensor.matmul(
                        pp[:],
                        wo_sb[:, kt * D + m8 * 128 : kt * D + (m8 + 1) * 128],
                        oT4[:, kt, n * 512 : (n + 1) * 512],
                        start=(kt == 0),
                        stop=(kt == 3),
                    )
                ost = wrk.tile([128, 512], f16, tag="ost", name="ost", bufs=6)
                nc.vector.tensor_copy(ost[:], pp[:])
                nc.sync.dma_start(
                    outT_d[m8 * 128 : (m8 + 1) * 128, n * 512 : (n + 1) * 512],
                    ost[:],
                )

            def outproj_unit_q(qt, m8, act_copy=False):
                # q-granular (N=128) variant so the last chunk's output
                # projection overlaps the remaining attention
                pp = ps.tile([128, 128], f32, tag="ps_small", name="pp",
                             bufs=2)
                for kt in range(4):
                    nc.tensor.matmul(
                        pp[:],
                        wo_sb[:, kt * D + m8 * 128 : kt * D + (m8 + 1) * 128],
                        oT4[:, kt, qt * 128 : (qt + 1) * 128],
                        start=(kt == 0),
                        stop=(kt == 3),
                    )
                ost = wrk.tile([128, 512], f16, tag="ost", name="ost", bufs=6)
                if act_copy:
                    nc.scalar.activation(ost[:, 0:128], pp[:], COPY)
                else:
                    nc.vector.tensor_copy(ost[:, 0:128], pp[:])
                nc.sync.dma_start(
                    outT_d[m8 * 128 : (m8 + 1) * 128,
                           qt * 128 : (qt + 1) * 128],
                    ost[:, 0:128],
                )

            def outproj_units(n):
                for m8 in range(8):
                    yield lambda m8=m8: outproj_unit(n, m8)

            # batch 0 prologue, DMA-priority ordered: the m=0 strips of
            # Wq/Wk plus the X chunks unlock attention(0, qt=0) ~10us in;
            # V next (the deferred mm2 tails need v5 tiles 0-3 by the
            # first full drain inside the m=1 group); remaining W strips
            # follow.  All 16 (hp, qt<4) attention calls interleave into
            # the m-strip loop -- they need only chunk-0 Q/K.
            load_w(wq_sb, wq_d, DM)
            nc.sync.dma_start(bq_sb[:], bq_d[:])
            nc.sync.dma_start(maskw_sb[:], maskw_d[:])
            nc.sync.dma_start(maskr_sb[:], maskr_d[:])
            nc.sync.dma_start(ident_sb[:], ident_d[:])
            xq = load_xch(xqT_d, 0, tag="xchq")
            proj_q_unit(xq, 0, 0)
            load_w(wk_sb, wk_d, DM)
            nc.sync.dma_start(bk_sb[:], bk_d[:])
            xk = load_xch(xkT_d, 0, tag="xchk")
            proj_k_unit(xk, 0, 0)

            # first score/exp right after the m=0 strips: only 16 proj
            # matmuls precede mm1(0,0) in PE order instead of 64
            o_nat0 = wrk.tile([128, DM], f16, tag="o_nat", name="o_nat",
                              bufs=8)
            attention(0, 0, o_nat0)
            for m in range(1, 4):
                proj_q_unit(xq, 0, m)
                proj_k_unit(xk, 0, m)
            load_w(wv_sb, wv_d, DM)
            nc.gpsimd.memset(v5[:, :, :, HD : HD + 1], 1.0)
            xv = load_xch(xvT_d, 0, nk=24, tag="xchv")
            for mi in range(4):
                fill_h32(xv, mi)
            load_w(wo_sb, wo_d, D)
            prefetch(1)
            proj_v_unit(xv, 0, 0)
            for m in range(1, 4):
                attention(m, 0, o_nat0)
            proj_v_unit(xv, 0, 1)
            proj_v_unit(xv, 0, 2)
            proj_v_unit(xv, 0, 3)
            pending.append(lambda: transpose_o(0, o_nat0))

            for n in range(4):
                # filler PE work interleaved into attention at call
                # granularity: previous chunks' output projection + next
                # chunk's projections
                fillers = []
                if n == 2:
                    fillers.extend(outproj_units(0))
                if n == 3:
                    fillers.extend(outproj_units(1))
                    fillers.extend(outproj_units(2))
                if n < 3:
                    fillers.extend(proj_units(n + 1))
                fillers = iter(fillers)
                for qt in range(4 * n, 4 * n + 4):
                    if qt == 0:
                        continue
                    last = qt == NQT - 1
                    o_nat = wrk.tile([128, DM], f16, tag="o_nat",
                                     name="o_nat", bufs=8)
                    for hp in range(4):
                        attention(hp, qt, o_nat,
                                  tail_split=(last and hp == 3))
                        u = next(fillers, None)
                        if u is not None:
                            u()
                    if not last:
                        pending.append(
                            lambda qt=qt, o=o_nat: transpose_o(qt, o))
                    else:
                        def transpose_pe(o=o_nat):
                            # low-latency on-PE transpose for the final
                            # tile: skips the XBAR DMA's HWDGE-gen + queue
                            # + sem chain (~2.8us) on the critical tail
                            pt = ps.tile([128, 512], f16, tag="ps_small",
                                         name="pt", bufs=2)
                            for mm in range(4):
                                nc.tensor.transpose(
                                    pt[:, mm * 128 : (mm + 1) * 128],
                                    o[:, mm * 128 : (mm + 1) * 128],
                                    ident_sb[:],
                                )
                            nc.scalar.activation(
                                oT4[:, :, (NQT - 1) * 128 : NQT * 128],
                                pt.rearrange("p (k s) -> p k s", k=4),
                                COPY,
                            )
                        pending.append(transpose_pe)
                    if n == 3 and qt > 12:
                        # for qt=15 hold back half of qt-14's outproj: it
                        # runs in the PE gap while the final transpose is
                        # in flight, keeping the PE clock ramped
                        for m8 in range(4 if last else 8):
                            outproj_unit_q(qt - 1, m8)
                for u in fillers:
                    u()
            flush_pending(1)           # tail_s: last mm2 block + norm
            # the held-back qt=14 strips run while the final norm completes,
            # keeping the PE clock ramped; they must precede the qt=15
            # transpose flush (its DMA write to oT_sb is tracked
            # conservatively and would stall any later oT reader)
            for m8 in range(4, 8):
                outproj_unit_q(14, m8)
            flush_pending()            # transpose(15)
            # final output strip: 8 psum groups -> one staging tile -> two
            # batched DMAs (two HWDGE generations instead of eight)
            qsl = slice(15 * 128, 16 * 128)
            outT_v = outT_d.rearrange("(k p) s -> p k s", p=128)
            ost_big = res.tile([128, 8 * 128], f16, tag="ost_big")
            ost_bv = ost_big.rearrange("p (k q) -> p k q", k=8)
            for m8 in range(8):
                pp = ps.tile([128, 128], f32, tag="ps_small", name="pp",
                             bufs=2)
                for kt in range(4):
                    nc.tensor.matmul(
                        pp[:],
                        wo_sb[:, kt * D + m8 * 128 : kt * D + (m8 + 1) * 128],
                        oT4[:, kt, qsl],
                        start=(kt == 0),
                        stop=(kt == 3),
                    )
                dst = ost_big[:, m8 * 128 : (m8 + 1) * 128]
                if m8 % 2 == 0:
                    nc.scalar.activation(dst, pp[:], COPY)
                else:
                    nc.vector.tensor_copy(dst, pp[:])
                if m8 == 3:
                    nc.sync.dma_start(outT_v[:, 0:4, qsl],
                                      ost_bv[:, 0:4, :])
            nc.sync.dma_start(outT_v[:, 4:8, qsl], ost_bv[:, 4:8, :])

    nc.compile()
    return nc


def _get_program():
    if "nc" not in _CACHE:
        _CACHE["nc"] = _build_program()
    return _CACHE["nc"]


def _make_in_maps(query, key, value, Wq, bq, Wk, bk, Wv, bv, Wo):
    import ml_dtypes

    f16 = np.float16
    e4 = ml_dtypes.float8_e4m3
    # maskw[p, t*128+k] = 1 if k == 64t+p ; maskr[p, t*128+q] = -240 if 64t+p > q
    # (-240 is the most negative finite float8e4 value; exp((s-240)/8) == 0)
    maskw = np.zeros((128, 256), dtype=np.float32)
    maskr = np.zeros((128, 256), dtype=np.float32)
    for p in range(64):
        for t in range(2):
            maskw[p, t * 128 + 64 * t + p] = 1.0
            kk = 64 * t + p
            maskr[p, t * 128 + 0 : t * 128 + min(kk, 128)] = -240.0
    # identical copy on partitions 64-127 for head 1's row strip
    maskw[64:128] = maskw[0:64]
    maskr[64:128] = maskr[0:64]
    maskw = maskw.astype(e4)
    maskr = maskr.astype(e4)
    ident = np.eye(128, dtype=f16)
    def hilo(x):
        h = x.astype(e4)
        l = (x - h.astype(np.float32)).astype(e4)
        return np.vstack([h, l])

    xq8 = [hilo(np.ascontiguousarray(query[b].T)) for b in range(B)]
    xk8 = [hilo(np.ascontiguousarray(key[b].T)) for b in range(B)]
    xv8 = [hilo(np.ascontiguousarray(value[b].T)) for b in range(B)]

    in_maps = []
    for c in range(NCORES):
        b, g = c // 2, c % 2
        sl = slice(g * DM, (g + 1) * DM)
        wv_h = Wv[:, sl].astype(e4)
        wv_l32 = (32.0 * (Wv[:, sl] - wv_h.astype(np.float32))).astype(e4)
        in_maps.append({
            "xqT": xq8[b],
            "xkT": xk8[b],
            "xvT": xv8[b],
            "wq": np.ascontiguousarray(Wq[:, sl]).astype(e4),
            "wk": np.ascontiguousarray(Wk[:, sl]).astype(e4),
            "wv": np.vstack([wv_h, wv_l32]),
            "bq": np.ascontiguousarray(bq[sl].reshape(4, 128).T.astype(np.float32)),
            "bk": np.ascontiguousarray(bk[sl].reshape(4, 128).T.astype(np.float32)),
            "wo": np.ascontiguousarray(Wo[sl, :]).astype(f16),
            "maskw": maskw,
            "maskr": maskr,
            "ident": ident,
        })
    return in_maps


def _run_spmd(in_maps, trace=False):
    from concourse import bass_utils

    nc = _get_program()
    return bass_utils.run_bass_kernel_spmd(
        nc, in_maps, core_ids=list(range(NCORES)), trace=trace
    )


def _assemble(res, bv, Wo, bo):
    out = np.empty((B, S, D), dtype=np.float32)
    bias = (bo.astype(np.float64) + bv.astype(np.float64) @ Wo.astype(np.float64)
            ).astype(np.float32)
    for b in range(B):
        out[b] = (
            res.results[2 * b]["outT"].astype(np.float32)
            + res.results[2 * b + 1]["outT"].astype(np.float32)
        ).T + bias
    return out


def _numpy_fallback(query, key, value, mask, Wq, bq, Wk, bk, Wv, bv, Wo, bo):
    """Correct (slow) host path for non-causal masks; never used when the
    mask is the reference's tril."""
    def split_heads(x):
        b, s, _ = x.shape
        return x.reshape(b, s, H, HD).transpose(0, 2, 1, 3)

    q = split_heads(query @ Wq + bq)
    k = split_heads(key @ Wk + bk)
    v = split_heads(value @ Wv + bv)
    nb = query.shape[0]
    out = np.empty((nb, H, S, HD), dtype=np.float32)
    for b in range(nb):
        mb = np.asarray(mask[b, 0]) != 0
        for h in range(H):
            s = (q[b, h] @ k[b, h].T) / math.sqrt(HD)
            s = np.where(mb, s, -np.inf)
            s -= s.max(axis=-1, keepdims=True)
            e = np.exp(s)
            a = e / e.sum(axis=-1, keepdims=True)
            a *= mb
            out[b, h] = a @ v[b, h]
    out = out.transpose(0, 2, 1, 3).reshape(nb, -1, D)
    return (out @ Wo + bo).astype(np.float32)


def kernel(query, key, value, mask, Wq, bq, Wk, bk, Wv, bv, Wo, bo):
    query = np.asarray(query, dtype=np.float32)
    key = np.asarray(key, dtype=np.float32)
    value = np.asarray(value, dtype=np.float32)
    mask = np.asarray(mask)
    Wq = np.asarray(Wq, dtype=np.float32)
    bq = np.asarray(bq, dtype=np.float32)
    Wk = np.asarray(Wk, dtype=np.float32)
    bk = np.asarray(bk, dtype=np.float32)
    Wv = np.asarray(Wv, dtype=np.float32)
    bv = np.asarray(bv, dtype=np.float32)
    Wo = np.asarray(Wo, dtype=np.float32)
    bo = np.asarray(bo, dtype=np.float32)

    causal = np.array_equal(
        np.asarray(mask[0, 0], dtype=np.int32),
        np.tril(np.ones((S, S), dtype=np.int32)),
    ) and all(np.array_equal(mask[b], mask[0]) for b in range(1, mask.shape[0]))
    if not causal:
        return _numpy_fallback(
            query, key, value, mask, Wq, bq, Wk, bk, Wv, bv, Wo, bo
        )

    in_maps = _make_in_maps(query, key, value, Wq, bq, Wk, bk, Wv, bv, Wo)
    res = _run_spmd(in_maps)
    return _assemble(res, bv, Wo, bo)



# revision 122
# speedup vs baseline: 1.0121x; 1.0121x over previous
"""Multi-head causal attention (B=4, S=2048, D=1024, H=16) on 8 trn2 NeuronCores.

Sharding: data-parallel over batch (4) x tensor-parallel over heads (2 groups
of 8).  Core c = (b, g) computes, for batch b, head group g:
  Projections run as fp8e4m3 DoubleRow matmuls (256 contraction rows per
  matmul at 0.5 cyc/output-elem = 4x fp16 PE throughput), with residual
  splits sized to the error budget (host precomputes all fp8 operands):
    Q^T = e4m3(Xh Wq8 + Xl Wq8 + bq)   2-term: X = Xh + Xl exact, Wq single
                                        fp8 (only W-quant noise ~3%)
    K^T likewise 2-term, then split into fp8 hi/lo pair (repr of K exact)
    V   = (Xh + Xl) Wv_h + Xh Wv_l32 / 32   3-term: separate psum group for
          the x32-scaled W-residual (W ~ +-1/32 underflows e4m3 unscaled),
          combined + cast f16 by one DVE scalar_tensor_tensor.  NO bias (bv
          contributes bv @ Wo, folded into the host-side bias).  A ones
          column is appended for softmax row sums.
  per head h, query tile qt (128):
     S^T[k, q] = (K_hi,K_lo)^T (Q̂,Q̂)   one fp8 DoubleRow matmul per k-tile
     diagonal tile: extra DR matmul accumulates -448*step[k,q] into psum so
       masked scores exp to exactly 0 (no post-exp masking pass)
     A^T = fp16(exp(S^T / 8))          (ScalarE, k-tiles packed in psum,
                                        one instruction covers both heads;
                                        for qt >= 9 the first 4-ktile block
                                        runs on DVE instead as an fp16
                                        Schraudolph bit-trick exp --
                                        bitcast_f16(int16(s*a + b)) -- whose
                                        ~1.8% centered noise the row
                                        normalization absorbs, offloading
                                        the otherwise-saturated ScalarE)
     psum_o[q, 65] += A^T.T V          fp16 matmuls, col 64 = row sums via
                                        the ones column
     O[q, h*64:+64] = psum_o[:, :64] * recip(psum_o[:, 64])  (one DVE
       tensor_tensor per head pair, broadcast recips, psum -> f16)
  O^T via SBUF-to-SBUF XBAR DMA transpose (the final q-tile instead uses
  an on-PE identity transpose + ScalarE copy, skipping the DMA HWDGE-gen
  + queue + sem latency on the critical tail), out^T = Wo_g^T O^T (fp16)
Host sums the two per-batch partials and adds bo + bv @ Wo.

Scheduling: each attention call's mm2+normalize is deferred until after
the NEXT call's first mm1/exp block so ScalarE/DVE stream exps without
waiting on PE tail work; X chunks prefetch one chunk ahead through
dedicated SBUF rings; the Xv/32 operand for the V lo-term is derived on
the idle Pool engine rather than transferred.
"""

import math

import numpy as np

B, S, D, H = 4, 2048, 1024, 16
HD = D // H          # 64
NCORES = 8
HPC = 8              # heads per core
DM = HPC * HD        # 512 mid-dims per core
NQT = S // 128       # 16 query tiles
KT_PER_EXP = 4       # k-tiles per head packed into one [128, 1024] psum before exp
VROW = 65            # per-head V columns incl. ones column

_CACHE = {}


def _build_program():
    import concourse.mybir as mybir
    import concourse.tile as tile
    from concourse import bacc

    f32 = mybir.dt.float32
    f16 = mybir.dt.float16
    i16 = mybir.dt.int16
    e4m = mybir.dt.float8e4
    SCH_QT = 9                       # offload exp blk0 to DVE for qt >= this
    SCH_A = 1024.0 / math.log(2.0) / 8.0
    SCH_B = 15360.0 - 60.0
    EXP = mybir.ActivationFunctionType.Exp
    IDENT = mybir.ActivationFunctionType.Identity
    COPY = mybir.ActivationFunctionType.Copy
    DR = mybir.MatmulPerfMode.DoubleRow
    ADD = mybir.AluOpType.add
    SUB = mybir.AluOpType.subtract
    MUL = mybir.AluOpType.mult

    nc = bacc.Bacc("TRN2", target_bir_lowering=False, debug=False,
                   num_devices=NCORES)

    xqT_d = nc.dram_tensor("xqT", [2 * D, S], e4m, kind="ExternalInput")
    xkT_d = nc.dram_tensor("xkT", [2 * D, S], e4m, kind="ExternalInput")
    xvT_d = nc.dram_tensor("xvT", [2 * D, S], e4m, kind="ExternalInput")
    wq_d = nc.dram_tensor("wq", [D, DM], e4m, kind="ExternalInput")
    wk_d = nc.dram_tensor("wk", [D, DM], e4m, kind="ExternalInput")
    wv_d = nc.dram_tensor("wv", [2 * D, DM], e4m, kind="ExternalInput")
    bq_d = nc.dram_tensor("bq", [128, 4], f32, kind="ExternalInput")
    bk_d = nc.dram_tensor("bk", [128, 4], f32, kind="ExternalInput")
    wo_d = nc.dram_tensor("wo", [DM, D], f16, kind="ExternalInput")
    maskw_d = nc.dram_tensor("maskw", [128, 256], e4m, kind="ExternalInput")
    maskr_d = nc.dram_tensor("maskr", [128, 256], e4m, kind="ExternalInput")
    ident_d = nc.dram_tensor("ident", [128, 128], f16, kind="ExternalInput")
    outT_d = nc.dram_tensor("outT", [D, S], f16, kind="ExternalOutput")

    with tile.TileContext(nc) as tc:
        with (
            tc.tile_pool(name="res", bufs=1) as res,     # long-lived tensors
            tc.tile_pool(name="wrk", bufs=1) as wrk,     # rotating work tiles
            tc.tile_pool(name="ps", bufs=1, space="PSUM") as ps,
        ):
            # ---- resident tensors -------------------------------------
            wq_sb = res.tile([128, 8 * DM], e4m, tag="wq_sb")
            wk_sb = res.tile([128, 8 * DM], e4m, tag="wk_sb")
            wv_sb = res.tile([128, 16 * DM], e4m, tag="wv_sb")
            wo_sb = res.tile([128, 4 * D], f16, tag="wo_sb")
            bq_sb = res.tile([128, 4], f32, tag="bq_sb")
            bk_sb = res.tile([128, 4], f32, tag="bk_sb")
            maskw_sb = res.tile([128, 256], e4m, tag="maskw_sb")
            maskr_sb = res.tile([128, 256], e4m, tag="maskr_sb")
            ident_sb = res.tile([128, 128], f16, tag="ident_sb")
            qT_sb = [res.tile([128, S], e4m, tag=f"qT{m}", name=f"qT{m}")
                     for m in range(4)]
            kT_sb = [res.tile([128, 2 * S], e4m, tag=f"kT{m}", name=f"kT{m}")
                     for m in range(4)]
            v_sb = res.tile([128, NQT * HPC * VROW], f16, tag="v_sb")
            oT_sb = res.tile([128, 4 * S], f16, tag="oT_sb")

            kT_v = [t.rearrange("p (t s) -> p t s", t=2) for t in kT_sb]
            # [p, ktile, head, hi/lo, col]
            v5 = v_sb.rearrange("p (s h c) -> p s h c", h=HPC, c=VROW)
            oT4 = oT_sb.rearrange("p (k s) -> p k s", k=4)
            # per-head views at the head's partition base: a psum accumulation
            # group whose matmuls sit on different row strips (score at base
            # 64, mask at base 0) wedges the device, so head 1 reads an
            # identical mask copy from partitions 64-127
            maskw_v = [maskw_sb[p0 : p0 + 64, :].rearrange("p (t k) -> p t k", t=2)
                       for p0 in (0, 64)]
            maskr_v = [maskr_sb[p0 : p0 + 64, :].rearrange("p (t k) -> p t k", t=2)
                       for p0 in (0, 64)]

            def load_w(w_sb, w_d, n_w, cols=None):
                sv = w_sb.rearrange("p (k n) -> p k n", n=n_w)
                dv = w_d.rearrange("(k p) n -> p k n", p=128)
                if cols is not None:
                    sv = sv[:, :, cols[0] : cols[1]]
                    dv = dv[:, :, cols[0] : cols[1]]
                nc.sync.dma_start(sv, dv)

            # ---- projections, split into per-m-tile units -------------
            # xch holds fp8 hi chunks (k=0..7) then lo chunks (k=8..15); DR
            # matmuls pair adjacent 128-row chunks (256 contraction rows per
            # matmul at 0.5 cyc/output-elem)
            def load_xch(xT_d, n, nk=16, tag="xch"):
                xch = wrk.tile([128, nk * 512], e4m, tag=tag, name=tag,
                               bufs=2)
                sv = xch.rearrange("p (k s) -> p k s", k=nk)
                dv = xT_d.rearrange("(k p) s -> p k s", p=128)[
                    :, :, n * 512 : (n + 1) * 512
                ]
                if nk == 24:
                    # V chunk: DMA only Xh/Xl (k 0-15); Xh/32 (k 16-23) is
                    # computed on the idle Pool engine, per mi strip
                    nc.sync.dma_start(sv[:, 0:16, :], dv[:, 0:16, :])
                else:
                    # hi chunks land first so the hi-group DR matmuls can
                    # start while the lo half still streams
                    nc.sync.dma_start(sv[:, 0:8, :], dv[:, 0:8, :])
                    nc.sync.dma_start(sv[:, 8:16, :], dv[:, 8:16, :])
                return xch

            def fill_h32(xch, mi):
                xv_ = xch.rearrange("p (k s) -> p k s", k=24)
                sl = slice(mi * 128, (mi + 1) * 128)
                nc.gpsimd.tensor_scalar_mul(
                    xv_[:, 16:24, sl], xv_[:, 0:8, sl], 1.0 / 32.0,
                )

            def proj_mm(xch, w_sb, m):
                # 2-term: (Xh + Xl) @ W8, one psum accumulation group
                pp = ps.tile([128, 512], f32, tag="ps_small", name="pp",
                             bufs=2)
                wv_ = w_sb.rearrange("p (k n) -> p k n", n=DM)
                xv_ = xch.rearrange("p (k s) -> p k s", k=16)
                lhs = wv_[:, :, m * 128 : (m + 1) * 128]
                idx = 0
                for base in (0, 8):
                    for j in range(4):
                        nc.tensor.matmul(
                            pp[:],
                            lhs[:, 2 * j : 2 * j + 2, :],
                            xv_[:, base + 2 * j : base + 2 * j + 2, :],
                            start=(idx == 0),
                            stop=(idx == 7),
                            perf_mode=DR,
                        )
                        idx += 1
                return pp

            def proj_q_unit(xch, n, m):
                pp = proj_mm(xch, wq_sb, m)
                nc.vector.tensor_scalar_add(
                    qT_sb[m][:, n * 512 : (n + 1) * 512], pp[:],
                    bq_sb[:, m : m + 1],
                )

            def proj_k_unit(xch, n, m, fast0=False):
                pp = proj_mm(xch, wk_sb, m)
                # fast0: split the bias-add + hi/lo pair so k-tile 0 (the
                # only tile attention(hp, 0) needs) is ready ~1.3us sooner
                # at kernel start
                cuts = ((0, 128), (128, 512)) if fast0 else ((0, 512),)
                for c0, c1 in cuts:
                    sl = slice(n * 512 + c0, n * 512 + c1)
                    nc.vector.tensor_scalar_add(
                        kT_v[m][:, 0, sl], pp[:, c0:c1], bk_sb[:, m : m + 1],
                    )
                    nc.vector.scalar_tensor_tensor(
                        kT_v[m][:, 1, sl], pp[:, c0:c1], bk_sb[:, m : m + 1],
                        kT_v[m][:, 0, sl], op0=ADD, op1=SUB,
                    )

            def proj_v_unit(xch, n, mi):
                st = n * 4 + mi          # S tile index
                # 3-term, one psum group: (Xh+Xl) Wh + (Xh/32)(32 Wl).
                # xch chunk layout: 0-7 Xh, 8-15 Xl, 16-23 Xh/32.
                pp = ps.tile([128, 512], f32, tag="ps_small", name="pp",
                             bufs=2)
                xv_ = xch.rearrange("p (k s) -> p k s", k=24)
                wvv = wv_sb.rearrange("p (k n) -> p k n", n=DM)
                lhs = xv_[:, :, mi * 128 : (mi + 1) * 128]
                idx = 0
                for base, wb in ((0, 0), (8, 0), (16, 8)):
                    for j in range(4):
                        nc.tensor.matmul(
                            pp[:],
                            lhs[:, base + 2 * j : base + 2 * j + 2, :],
                            wvv[:, wb + 2 * j : wb + 2 * j + 2, :],
                            start=(idx == 0),
                            stop=(idx == 11),
                            perf_mode=DR,
                        )
                        idx += 1
                pp3 = pp.rearrange("p (h c) -> p h c", h=HPC)
                nc.vector.tensor_copy(v5[:, st, :, 0:HD], pp3)

            preloaded = {}

            def prefetch(n):
                if n > 3 or n in preloaded:
                    return
                preloaded[n] = (load_xch(xqT_d, n, tag="xchq"),
                                load_xch(xkT_d, n, tag="xchk"),
                                load_xch(xvT_d, n, nk=24, tag="xchv"))

            def proj_units(n):
                xq, xk, xv = preloaded.pop(n)
                prefetch(n + 1)
                for m in range(4):
                    yield lambda m=m, x=xq: proj_q_unit(x, n, m)
                for m in range(4):
                    yield lambda m=m, x=xk: proj_k_unit(x, n, m)
                for mi in range(4):
                    fill_h32(xv, mi)
                for mi in range(4):
                    yield lambda mi=mi, x=xv: proj_v_unit(x, n, mi)

            # ---- attention -------------------------------------------
            # pending: deferred tails (last mm2 block + normalize, and
            # transpose_o) consumed right after the NEXT attention call's
            # first mm1/exp block is issued, so ScalarE streams exps
            # back-to-back instead of waiting for PE tail work
            pending = []

            def flush_pending(limit=None):
                n = len(pending) if limit is None else min(limit, len(pending))
                for _ in range(n):
                    pending.pop(0)()

            def attention(hp, qt, o_nat, drain=None, inline=False,
                          tail_split=False, local=False):
                nblk = qt + 1
                heads = (2 * hp, 2 * hp + 1)
                aT = wrk.tile([128, 2 * S], f16, tag="aT", name="aT", bufs=6)
                aTv = aT.rearrange("p (h s q) -> p h s q", h=2, q=128)
                po = ps.tile([128, 1024], f32, tag="ps_o", name="po", bufs=1)
                pof = [po[:, 0:VROW], po[:, 512 : 512 + VROW]]

                def mm1_exp(blk, cnt):
                    psx = ps.tile([128, 2 * KT_PER_EXP * 128], f32,
                                  tag="ps_s", name="psx", bufs=2)
                    for j in range(cnt):
                        kt = blk * KT_PER_EXP + j
                        diag = kt == qt
                        for hh in range(2):
                            p0 = hh * 64
                            rhs = qT_sb[hp][
                                p0 : p0 + 64, qt * 128 : (qt + 1) * 128
                            ].unsqueeze(1).broadcast_to([64, 2, 128])
                            dst = psx[:, hh * KT_PER_EXP * 128 + j * 128 :
                                      hh * KT_PER_EXP * 128 + (j + 1) * 128]
                            nc.tensor.matmul(
                                dst,
                                kT_v[hp][p0 : p0 + 64, :,
                                         kt * 128 : (kt + 1) * 128],
                                rhs,
                                start=True,
                                stop=not diag,
                                perf_mode=DR,
                                skip_group_check=True,
                            )
                            if diag:
                                nc.tensor.matmul(
                                    dst,
                                    maskw_v[hh],
                                    maskr_v[hh],
                                    start=False,
                                    stop=True,
                                    perf_mode=DR,
                                    skip_group_check=True,
                                )
                    psxv = psx.rearrange("p (h j q) -> p h j q", h=2, q=128)
                    ksl = slice(blk * KT_PER_EXP, blk * KT_PER_EXP + cnt)
                    if blk == 0 and qt >= SCH_QT:
                        # far-from-diagonal k-tiles: fp16 Schraudolph exp on
                        # DVE (bitcast_f16(int16(s*a + b)) ~ exp(s/8), 1.8%
                        # centered noise, absorbed by the row normalization)
                        # to offload the otherwise-saturated ScalarE
                        aTi = aT[:].bitcast(i16).rearrange(
                            "p (h s q) -> p h s q", h=2, q=128)
                        with nc.allow_low_precision(reason="schraudolph exp"):
                            nc.vector.tensor_scalar(
                                aTi[:, :, ksl, :], psxv[:, :, 0:cnt, :],
                                SCH_A, SCH_B, op0=MUL, op1=ADD,
                            )
                    else:
                        nc.scalar.activation(
                            aTv[:, :, ksl, :],
                            psxv[:, :, 0:cnt, :],
                            EXP,
                            scale=1.0 / math.sqrt(HD),
                        )

                def mm2(blk, cnt):
                    for hh in range(2):
                        for j in range(cnt):
                            kt = blk * KT_PER_EXP + j
                            nc.tensor.matmul(
                                pof[hh],
                                aTv[:, hh, kt, :],
                                v5[:, kt, heads[hh], :],
                                start=(kt == 0),
                                stop=(kt == nblk - 1),
                                skip_group_check=True,
                            )

                nexp = (nblk + KT_PER_EXP - 1) // KT_PER_EXP
                cnts = [min(KT_PER_EXP, nblk - b * KT_PER_EXP)
                        for b in range(nexp)]
                for blk in range(nexp):
                    mm1_exp(blk, cnts[blk])
                    if blk == min(1, nexp - 1):
                        flush_pending(drain)
                    if inline and blk > 0:
                        mm2(blk - 1, cnts[blk - 1])

                def norm():
                    rc = wrk.tile([128, 2], f32, tag="rc", name="rc", bufs=12)
                    po_pair = po.rearrange("p (b c) -> p b c", c=512)
                    nc.vector.reciprocal(rc[:], po_pair[:, :, HD : HD + 1])
                    rcb = rc[:].unsqueeze(2).broadcast_to([128, 2, HD])
                    if local:
                        o_pair = o_nat.rearrange("p (h c) -> p h c", h=2)
                    else:
                        o_pair = o_nat.rearrange("p (h c) -> p h c", h=HPC)[
                            :, 2 * hp : 2 * hp + 2, :
                        ]
                    nc.vector.tensor_tensor(
                        o_pair, po_pair[:, :, 0:HD], rcb, op=MUL,
                    )

                def strip_transpose():
                    # per-head-pair transpose from a PER-CALL [128,128]
                    # tile (separate tiles avoid the conservative DMA-read
                    # hazard that would stall the other head-pairs' norms)
                    src_ap = o_nat[:] if local else \
                        o_nat[:, hp * 128 : (hp + 1) * 128]
                    nc.sync.dma_start_transpose(
                        oT4[:, hp, qt * 128 : (qt + 1) * 128], src_ap,
                    )

                if inline:
                    mm2(nexp - 1, cnts[nexp - 1])
                    norm()
                    strip_transpose()
                    return

                if tail_split:
                    # very last call: no future exps to protect, so run all
                    # but the final mm2 block now; only the last block +
                    # norm stay gated on the final exp
                    for blk in range(nexp - 1):
                        mm2(blk, cnts[blk])

                    def tail_s():
                        mm2(nexp - 1, cnts[nexp - 1])
                        norm()

                    pending.append(tail_s)
                    return

                def tail():
                    for blk in range(nexp):
                        mm2(blk, cnts[blk])
                    norm()

                pending.append(tail)

            def transpose_o(qt, o_nat):
                # SBUF->SBUF XBAR transpose: oT4[d, k, q] = o_nat[q, k*128+d]
                nc.sync.dma_start_transpose(
                    oT4[:, :, qt * 128 : (qt + 1) * 128], o_nat[:]
                )

            def outproj_unit(n, m8):
                pp = ps.tile([128, 512], f32, tag="ps_small", name="pp",
                             bufs=2)
                for kt in range(4):
                    nc.tensor.matmul(
                        pp[:],
                        wo_sb[:, kt * D + m8 * 128 : kt * D + (m8 + 1) * 128],
                        oT4[:, kt, n * 512 : (n + 1) * 512],
                        start=(kt == 0),
                        stop=(kt == 3),
                    )
                ost = wrk.tile([128, 512], f16, tag="ost", name="ost", bufs=6)
                nc.vector.tensor_copy(ost[:], pp[:])
                nc.sync.dma_start(
                    outT_d[m8 * 128 : (m8 + 1) * 128, n * 512 : (n + 1) * 512],
                    ost[:],
                )

            def outproj_unit_q(qt, m8, act_copy=False):
                # q-granular (N=128) variant so the last chunk's output
                # projection overlaps the remaining attention
                pp = ps.tile([128, 128], f32, tag="ps_small", name="pp",
                             bufs=2)
                for kt in range(4):
                    nc.tensor.matmul(
                        pp[:],
                        wo_sb[:, kt * D + m8 * 128 : kt * D + (m8 + 1) * 128],
                        oT4[:, kt, qt * 128 : (qt + 1) * 128],
                        start=(kt == 0),
                        stop=(kt == 3),
                    )
                ost = wrk.tile([128, 512], f16, tag="ost", name="ost", bufs=6)
                if act_copy:
                    nc.scalar.activation(ost[:, 0:128], pp[:], COPY)
                else:
                    nc.vector.tensor_copy(ost[:, 0:128], pp[:])
                nc.sync.dma_start(
                    outT_d[m8 * 128 : (m8 + 1) * 128,
                           qt * 128 : (qt + 1) * 128],
                    ost[:, 0:128],
                )

            def outproj_units(n):
                for m8 in range(8):
                    yield lambda m8=m8: outproj_unit(n, m8)

            # batch 0 prologue, DMA-priority ordered: the m=0 strips of
            # Wq/Wk plus the X chunks unlock attention(0, qt=0) ~10us in;
            # V next (the deferred mm2 tails need v5 tiles 0-3 by the
            # first full drain inside the m=1 group); remaining W strips
            # follow.  All 16 (hp, qt<4) attention calls interleave into
            # the m-strip loop -- they need only chunk-0 Q/K.
            load_w(wq_sb, wq_d, DM)
            nc.sync.dma_start(bq_sb[:], bq_d[:])
            nc.sync.dma_start(maskw_sb[:], maskw_d[:])
            nc.sync.dma_start(maskr_sb[:], maskr_d[:])
            nc.sync.dma_start(ident_sb[:], ident_d[:])
            xq = load_xch(xqT_d, 0, tag="xchq")
            proj_q_unit(xq, 0, 0)
            load_w(wk_sb, wk_d, DM)
            nc.sync.dma_start(bk_sb[:], bk_d[:])
            xk = load_xch(xkT_d, 0, tag="xchk")
            proj_k_unit(xk, 0, 0)

            # first score/exp right after the m=0 strips: only 16 proj
            # matmuls precede mm1(0,0) in PE order instead of 64
            o_nat0 = wrk.tile([128, DM], f16, tag="o_nat", name="o_nat",
                              bufs=8)
            attention(0, 0, o_nat0)
            for m in range(1, 4):
                proj_q_unit(xq, 0, m)
                proj_k_unit(xk, 0, m)
            load_w(wv_sb, wv_d, DM)
            nc.gpsimd.memset(v5[:, :, :, HD : HD + 1], 1.0)
            xv = load_xch(xvT_d, 0, nk=24, tag="xchv")
            for mi in range(4):
                fill_h32(xv, mi)
            load_w(wo_sb, wo_d, D)
            prefetch(1)
            proj_v_unit(xv, 0, 0)
            for m in range(1, 4):
                attention(m, 0, o_nat0)
            proj_v_unit(xv, 0, 1)
            proj_v_unit(xv, 0, 2)
            proj_v_unit(xv, 0, 3)
            pending.append(lambda: transpose_o(0, o_nat0))

            for n in range(4):
                # filler PE work interleaved into attention at call
                # granularity: previous chunks' output projection + next
                # chunk's projections
                fillers = []
                if n == 2:
                    fillers.extend(outproj_units(0))
                if n == 3:
                    fillers.extend(outproj_units(1))
                    fillers.extend(outproj_units(2))
                if n < 3:
                    fillers.extend(proj_units(n + 1))
                fillers = iter(fillers)
                for qt in range(4 * n, 4 * n + 4):
                    if qt == 0:
                        continue
                    last = qt == NQT - 1
                    o_nat = wrk.tile([128, DM], f16, tag="o_nat",
                                     name="o_nat", bufs=8)
                    for hp in range(4):
                        attention(hp, qt, o_nat,
                                  tail_split=(last and hp == 3))
                        u = next(fillers, None)
                        if u is not None:
                            u()
                    if not last:
                        pending.append(
                            lambda qt=qt, o=o_nat: transpose_o(qt, o))
                    else:
                        def transpose_pe(o=o_nat):
                            # low-latency on-PE transpose for the final
                            # tile: skips the XBAR DMA's HWDGE-gen + queue
                            # + sem chain (~2.8us) on the critical tail
                            pt = ps.tile([128, 512], f16, tag="ps_small",
                                         name="pt", bufs=2)
                            for mm in range(4):
                                nc.tensor.transpose(
                                    pt[:, mm * 128 : (mm + 1) * 128],
                                    o[:, mm * 128 : (mm + 1) * 128],
                                    ident_sb[:],
                                )
                            nc.scalar.activation(
                                oT4[:, :, (NQT - 1) * 128 : NQT * 128],
                                pt.rearrange("p (k s) -> p k s", k=4),
                                COPY,
                            )
                        pending.append(transpose_pe)
                    if n == 3 and qt > 12:
                        # for qt=15 hold back half of qt-14's outproj: it
                        # runs in the PE gap while the final transpose is
                        # in flight, keeping the PE clock ramped
                        for m8 in range(4 if last else 8):
                            outproj_unit_q(qt - 1, m8)
                for u in fillers:
                    u()
            flush_pending(1)           # tail_s: last mm2 block + norm
            # the held-back qt=14 strips run while the final norm completes,
            # keeping the PE clock ramped; they must precede the qt=15
            # transpose flush (its DMA write to oT_sb is tracked
            # conservatively and would stall any later oT reader)
            for m8 in range(4, 8):
                outproj_unit_q(14, m8)
            flush_pending()            # transpose(15)
            # final output strip: 8 psum groups -> one staging tile -> two
            # batched DMAs (two HWDGE generations instead of eight)
            qsl = slice(15 * 128, 16 * 128)
            outT_v = outT_d.rearrange("(k p) s -> p k s", p=128)
            ost_big = res.tile([128, 8 * 128], f16, tag="ost_big")
            ost_bv = ost_big.rearrange("p (k q) -> p k q", k=8)
            for m8 in range(8):
                pp = ps.tile([128, 128], f32, tag="ps_small", name="pp",
                             bufs=2)
                for kt in range(4):
                    nc.tensor.matmul(
                        pp[:],
                        wo_sb[:, kt * D + m8 * 128 : kt * D + (m8 + 1) * 128],
                        oT4[:, kt, qsl],
                        start=(kt == 0),
                        stop=(kt == 3),
                    )
                dst = ost_big[:, m8 * 128 : (m8 + 1) * 128]
                if m8 % 2 == 0:
                    nc.scalar.activation(dst, pp[:], COPY)
                else:
                    nc.vector.tensor_copy(dst, pp[:])
                if m8 == 3:
                    nc.sync.dma_start(outT_v[:, 0:4, qsl],
                                      ost_bv[:, 0:4, :])
            nc.sync.dma_start(outT_v[:, 4:8, qsl], ost_bv[:, 4:8, :])

    nc.compile()
    return nc


def _get_program():
    if "nc" not in _CACHE:
        _CACHE["nc"] = _build_program()
    return _CACHE["nc"]


def _make_in_maps(query, key, value, Wq, bq, Wk, bk, Wv, bv, Wo):
    import ml_dtypes

    f16 = np.float16
    e4 = ml_dtypes.float8_e4m3
    # maskw[p, t*128+k] = 1 if k == 64t+p ; maskr[p, t*128+q] = -240 if 64t+p > q
    # (-240 is the most negative finite float8e4 value; exp((s-240)/8) == 0)
    maskw = np.zeros((128, 256), dtype=np.float32)
    maskr = np.zeros((128, 256), dtype=np.float32)
    for p in range(64):
        for t in range(2):
            maskw[p, t * 128 + 64 * t + p] = 1.0
            kk = 64 * t + p
            maskr[p, t * 128 + 0 : t * 128 + min(kk, 128)] = -240.0
    # identical copy on partitions 64-127 for head 1's row strip
    maskw[64:128] = maskw[0:64]
    maskr[64:128] = maskr[0:64]
    maskw = maskw.astype(e4)
    maskr = maskr.astype(e4)
    ident = np.eye(128, dtype=f16)
    def hilo(x):
        h = x.astype(e4)
        l = (x - h.astype(np.float32)).astype(e4)
        return np.vstack([h, l])

    xq8 = [hilo(np.ascontiguousarray(query[b].T)) for b in range(B)]
    xk8 = [hilo(np.ascontiguousarray(key[b].T)) for b in range(B)]
    xv8 = [hilo(np.ascontiguousarray(value[b].T)) for b in range(B)]

    in_maps = []
    for c in range(NCORES):
        b, g = c // 2, c % 2
        sl = slice(g * DM, (g + 1) * DM)
        wv_h = Wv[:, sl].astype(e4)
        wv_l32 = (32.0 * (Wv[:, sl] - wv_h.astype(np.float32))).astype(e4)
        in_maps.append({
            "xqT": xq8[b],
            "xkT": xk8[b],
            "xvT": xv8[b],
            "wq": np.ascontiguousarray(Wq[:, sl]).astype(e4),
            "wk": np.ascontiguousarray(Wk[:, sl]).astype(e4),
            "wv": np.vstack([wv_h, wv_l32]),
            "bq": np.ascontiguousarray(bq[sl].reshape(4, 128).T.astype(np.float32)),
            "bk": np.ascontiguousarray(bk[sl].reshape(4, 128).T.astype(np.float32)),
            "wo": np.ascontiguousarray(Wo[sl, :]).astype(f16),
            "maskw": maskw,
            "maskr": maskr,
            "ident": ident,
        })
    return in_maps


def _run_spmd(in_maps, trace=False):
    from concourse import bass_utils

    nc = _get_program()
    return bass_utils.run_bass_kernel_spmd(
        nc, in_maps, core_ids=list(range(NCORES)), trace=trace
    )


def _assemble(res, bv, Wo, bo):
    out = np.empty((B, S, D), dtype=np.float32)
    bias = (bo.astype(np.float64) + bv.astype(np.float64) @ Wo.astype(np.float64)
            ).astype(np.float32)
    for b in range(B):
        out[b] = (
            res.results[2 * b]["outT"].astype(np.float32)
            + res.results[2 * b + 1]["outT"].astype(np.float32)
        ).T + bias
    return out


def _numpy_fallback(query, key, value, mask, Wq, bq, Wk, bk, Wv, bv, Wo, bo):
    """Correct (slow) host path for non-causal masks; never used when the
    mask is the reference's tril."""
    def split_heads(x):
        b, s, _ = x.shape
        return x.reshape(b, s, H, HD).transpose(0, 2, 1, 3)

    q = split_heads(query @ Wq + bq)
    k = split_heads(key @ Wk + bk)
    v = split_heads(value @ Wv + bv)
    nb = query.shape[0]
    out = np.empty((nb, H, S, HD), dtype=np.float32)
    for b in range(nb):
        mb = np.asarray(mask[b, 0]) != 0
        for h in range(H):
            s = (q[b, h] @ k[b, h].T) / math.sqrt(HD)
            s = np.where(mb, s, -np.inf)
            s -= s.max(axis=-1, keepdims=True)
            e = np.exp(s)
            a = e / e.sum(axis=-1, keepdims=True)
            a *= mb
            out[b, h] = a @ v[b, h]
    out = out.transpose(0, 2, 1, 3).reshape(nb, -1, D)
    return (out @ Wo + bo).astype(np.float32)


def kernel(query, key, value, mask, Wq, bq, Wk, bk, Wv, bv, Wo, bo):
    query = np.asarray(query, dtype=np.float32)
    key = np.asarray(key, dtype=np.float32)
    value = np.asarray(value, dtype=np.float32)
    mask = np.asarray(mask)
    Wq = np.asarray(Wq, dtype=np.float32)
    bq = np.asarray(bq, dtype=np.float32)
    Wk = np.asarray(Wk, dtype=np.float32)
    bk = np.asarray(bk, dtype=np.float32)
    Wv = np.asarray(Wv, dtype=np.float32)
    bv = np.asarray(bv, dtype=np.float32)
    Wo = np.asarray(Wo, dtype=np.float32)
    bo = np.asarray(bo, dtype=np.float32)

    causal = np.array_equal(
        np.asarray(mask[0, 0], dtype=np.int32),
        np.tril(np.ones((S, S), dtype=np.int32)),
    ) and all(np.array_equal(mask[b], mask[0]) for b in range(1, mask.shape[0]))
    if not causal:
        return _numpy_fallback(
            query, key, value, mask, Wq, bq, Wk, bk, Wv, bv, Wo, bo
        )

    in_maps = _make_in_maps(query, key, value, Wq, bq, Wk, bk, Wv, bv, Wo)
    res = _run_spmd(in_maps)
    return _assemble(res, bv, Wo, bo)

